# revision 44
# baseline (speedup 1.0000x reference)
"""GPTNeoX attention (B=1, S=2048, E=1024, 16 heads, hs=64) on 8 TRN2 cores.

Sharding: tensor-parallel across heads, 2 heads per core; host sums the 8
partial output projections (the all-reduce) and adds b_dense.

Perf notes (fp32 baseline 272us -> 156us -> this):
 - Matmuls bf16 except scores, which run fp8e4m3 DoubleRow: k is split
   hi+lo (lo = exact residual of the hi cast) across the two DR slots so
   only q carries fp8 noise (~1.1e-2 full-output rel err vs 2e-2 tol).
   DR halves the scores streaming time (0.5 cyc/col).
 - rotary folded into W_q on the host (W_q.T @ rot).
 - b_v folded into V before the PV matmul: P@(v+bv) = y_un + Z*bv, so the
   normalize (y_un + Z*bv)/Z = y + bv needs no separate bias pass.
 - softmax denominator Z from a ones-column appended to V (row 64 of the
   PV accumulator); 1/Z on DVE (plain reciprocal: the custom-DVE
   reciprocal_approx_* ucode returns garbage on this HW runtime) and the
   column broadcast via gpsimd partition_broadcast (works on HW; beware
   gpsimd cannot READ PSUM - that kills the NEFF build).
 - steady-state j-loop is EXP-BOUND (scalar ACTIVATE [128,1024] = 1.11us
   + ~0.1us sem = 1.21us/iter; PE needs ~1.07us). Everything else is
   arranged to keep the exp stream dense:
    * q/k projections first (attention starts right after the shorter v
      projection; the PE executes strictly in program order so attention
      can only start after ALL projection matmuls).
    * V transposed per (head, S-half) so vaug[h0, first half] is ready
      before the first PV needs it.
    * pt pool is 6 deep so exp can run ahead of a briefly-stalled PV.
    * output-projection block pairs interleave every 3rd j, split over
      both head loops of the next chunk (PE slack per j is only ~140ns).
    * the kernel tail normalizes the last head in 4 pipelined splits
      (small first - the first reciprocal gates the outproj tail).
 - sim-pass/HW-fail traps hit during development: reciprocal_approx_fast
   (garbage on HW), gpsimd reads of PSUM (NEFF build error), scalar-queue
   dma_start_transpose (corrupt data), gpsimd dma_start of big tiles
   (wrong results).  Matmul outputs cannot cross a PSUM bank (512 fp32).
"""

import numpy as np
import ml_dtypes

import concourse.bass as bass
import concourse.mybir as mybir
import concourse.tile as tile
from concourse import bacc
from concourse.bass_utils import run_bass_kernel_spmd

FP = mybir.dt.float32
BF = mybir.dt.bfloat16
F8 = mybir.dt.float8e4
AF = mybir.ActivationFunctionType
DR = mybir.MatmulPerfMode.DoubleRow
ALU = mybir.AluOpType

N_CORES = 8
E = 1024          # embed dim
S = 2048          # sequence
P = 128           # partitions
EO = E // P       # 8 e-chunks
HS = 64           # head size
NH_LOC = 2        # heads per core
SQB = 1024        # sq block (exp tile width, PSUM tile width)
NQB = S // SQB    # 2
SKC = S // P      # 16 sk chunks
NSC = S // P      # 16 s chunks for output


def build_nc():
    nc = bacc.Bacc("TRN2", target_bir_lowering=False, debug=False)

    xT_d = nc.dram_tensor("xT", (E, S), BF, kind="ExternalInput")
    wqT_d = nc.dram_tensor("wqT", (E, P), BF, kind="ExternalInput")
    wkT_d = nc.dram_tensor("wkT", (E, P), BF, kind="ExternalInput")
    wvT_d = nc.dram_tensor("wvT", (E, P), BF, kind="ExternalInput")
    wdT_d = nc.dram_tensor("wdT", (P, E), BF, kind="ExternalInput")
    bqe_d = nc.dram_tensor("bqe", (P,), FP, kind="ExternalInput")
    bk_d = nc.dram_tensor("bk", (P,), FP, kind="ExternalInput")
    bv_d = nc.dram_tensor("bv", (P,), FP, kind="ExternalInput")
    out_d = nc.dram_tensor("out", (S, E), BF, kind="ExternalOutput")

    xT_r = xT_d[:].rearrange("(eo p) s -> p eo s", p=P)
    wqT_r = wqT_d[:].rearrange("(eo p) g -> p eo g", p=P)
    wkT_r = wkT_d[:].rearrange("(eo p) g -> p eo g", p=P)
    wvT_r = wvT_d[:].rearrange("(eo p) g -> p eo g", p=P)

    with tile.TileContext(nc) as tc:
        with (
            nc.allow_low_precision(reason="bf16/fp8 matmul path; tol is 2e-2"),
            tc.tile_pool(name="const", bufs=1) as const,
            tc.tile_pool(name="work", bufs=2) as work,
            tc.tile_pool(name="ptp", bufs=6) as ptp,
            tc.tile_pool(name="nrm", bufs=2) as nrm,
            tc.tile_pool(name="outp", bufs=3) as outp,
            tc.tile_pool(name="psA", bufs=2, space="PSUM") as psA,
            tc.tile_pool(name="psB", bufs=2, space="PSUM") as psB,
        ):
            # ---------- constant loads ----------
            # q/k weights first (q/k projections run first); xT split
            # across the sync+scalar HWDGE queues (gpsimd DMA gives wrong
            # results on HW for these transfers).
            wkT_sb = const.tile([P, EO, P], BF)
            nc.scalar.dma_start(wkT_sb[:], wkT_r[:])
            wqT_sb = const.tile([P, EO, P], BF)
            nc.scalar.dma_start(wqT_sb[:], wqT_r[:])
            xT_sb = const.tile([P, EO, S], BF)
            nc.sync.dma_start(xT_sb[:, 0, :1024], xT_r[:, 0, :1024])
            nc.sync.dma_start(xT_sb[:, 0, 1024:], xT_r[:, 0, 1024:])
            nc.scalar.dma_start(xT_sb[:, 1, :], xT_r[:, 1, :])
            nc.sync.dma_start(xT_sb[:, 2, :], xT_r[:, 2, :])
            nc.scalar.dma_start(xT_sb[:, 3, :], xT_r[:, 3, :])
            nc.sync.dma_start(xT_sb[:, 4, :], xT_r[:, 4, :])
            nc.scalar.dma_start(xT_sb[:, 5, :], xT_r[:, 5, :])
            nc.sync.dma_start(xT_sb[:, 6, :], xT_r[:, 6, :])
            nc.scalar.dma_start(xT_sb[:, 7, :], xT_r[:, 7, :])
            wvT_sb = const.tile([P, EO, P], BF)
            nc.scalar.dma_start(wvT_sb[:], wvT_r[:])
            wdT_sb = const.tile([P, E], BF)
            nc.scalar.dma_start(wdT_sb[:], wdT_d[:])
            bqe_sb = const.tile([P, 1], FP)
            nc.scalar.dma_start(bqe_sb[:], bqe_d[:][:, None])
            bk_sb = const.tile([P, 1], FP)
            nc.scalar.dma_start(bk_sb[:], bk_d[:][:, None])
            bv_sb = const.tile([P, 1], FP)
            nc.scalar.dma_start(bv_sb[:], bv_d[:][:, None])

            vaug_sb = const.tile([P, NH_LOC, SKC, HS + 1], BF)
            nc.gpsimd.memset(vaug_sb[:, :, :, HS:HS + 1], 1.0)

            # q/k live as fp8e4m3 planes for the DoubleRow scores matmul:
            # kF plane0 = hi cast, plane1 = exact lo residual; qF both
            # planes hold the same q8 (DR slots sum k_hi*q8 + k_lo*q8).
            qF_sb = const.tile([P, 2, S], F8)
            kF_sb = const.tile([P, 2, S], F8)
            vT_sb = const.tile([P, S], BF)
            yTn_sb = const.tile([P, S], BF)

            # ---------- phase 2: k/q projections + fp8 casts -------------
            for half in range(2):
                base = half * (S // 2)
                tk = psB.tile([P, SQB], FP, tag="yt")
                tq = psB.tile([P, SQB], FP, tag="yt")
                for ec in range(EO):
                    for (t, w) in ((tk, wkT_sb), (tq, wqT_sb)):
                        for r in range(2):
                            nc.tensor.matmul(
                                t[:, r * 512:(r + 1) * 512],
                                lhsT=w[:, ec, :],
                                rhs=xT_sb[:, ec, base + r * 512:
                                          base + (r + 1) * 512],
                                start=(ec == 0),
                                stop=(ec == EO - 1),
                            )
                for r in range(2):
                    sl = slice(base + r * 512, base + (r + 1) * 512)
                    rs = slice(r * 512, (r + 1) * 512)
                    nc.vector.tensor_scalar_add(
                        kF_sb[:, 0, sl], tk[:, rs], bk_sb[:])
                    nc.vector.scalar_tensor_tensor(
                        kF_sb[:, 1, sl], tk[:, rs], bk_sb[:],
                        kF_sb[:, 0, sl], op0=ALU.add, op1=ALU.subtract)
                    nc.vector.tensor_scalar_add(
                        qF_sb[:, 0, sl], tq[:, rs], bqe_sb[:])
                    nc.gpsimd.tensor_copy(
                        qF_sb[:, 1, sl], qF_sb[:, 0, sl])

            # ---------- phase 1: v projection (+b_v), transpose ----------
            # vT[g,s] = sum_e wvT[e,g] xT[e,s] + bv[g]; then DMA-transpose
            # 64x128 head-blocks into vaug[sk, d] (ones col preset above).
            for half in range(2):
                base = half * (S // 2)
                tv = psA.tile([P, SQB], FP, tag="st")
                for ec in range(EO):
                    for r in range(2):
                        nc.tensor.matmul(
                            tv[:, r * 512:(r + 1) * 512],
                            lhsT=wvT_sb[:, ec, :],
                            rhs=xT_sb[:, ec, base + r * 512:
                                      base + (r + 1) * 512],
                            start=(ec == 0),
                            stop=(ec == EO - 1),
                        )
                for r in range(2):
                    sl = slice(base + r * 512, base + (r + 1) * 512)
                    nc.vector.tensor_scalar_add(
                        vT_sb[:, sl], tv[:, r * 512:(r + 1) * 512], bv_sb[:])
            # transpose per (head, S-half): the half0 block only needs the
            # first half of vT, so vaug[h0, j<8] is ready ~4us earlier and
            # the first PV doesn't stall the exp stream.
            for h in range(NH_LOC):
                hsl = slice(h * HS, (h + 1) * HS)
                for vh in range(2):
                    ssl = slice(vh * (S // 2), (vh + 1) * (S // 2))
                    ksl = slice(vh * (SKC // 2), (vh + 1) * (SKC // 2))
                    vstg = work.tile([P, SKC // 2, HS], BF, tag="vstg")
                    nc.sync.dma_start_transpose(vstg[:], vT_sb[hsl, ssl])
                    nc.vector.tensor_copy(vaug_sb[:, h, ksl, :HS], vstg[:])

            # ---------- attention ----------
            # ST[sk,sq] = (k_hi+k_lo) q8^T / 8 (fp8 DR) -> P~ = exp
            # yt = [V+bv | 1]^T P~ ; y = yt[:64]/Z with Z = yt[64].
            # out[s,f] = sum_e yTn[e,s] wdT[e,f]: outproj block pairs are
            # interleaved into the NEXT chunk's j-loops (both heads).
            def emit_po(sc, tail=False):
                po = psA.tile([P, SQB], FP, tag="st")
                for r in range(2):
                    rsl = slice(r * 512, (r + 1) * 512)
                    nc.tensor.matmul(
                        po[:, rsl],
                        lhsT=yTn_sb[:, sc * P:(sc + 1) * P],
                        rhs=wdT_sb[:, rsl],
                        start=True,
                        stop=True,
                    )
                ob = outp.tile([P, E], BF, tag="ob")
                if sc % 2 == 0:
                    nc.scalar.copy(ob[:], po[:])
                else:
                    nc.vector.tensor_copy(ob[:], po[:])
                nc.sync.dma_start(out_d[sc * P:(sc + 1) * P, :], ob[:])

            chunks = [(0, 1024), (1024, 1024)]
            prev_po = []
            for (cq0, csz) in chunks:
                qsl = slice(cq0, cq0 + csz)
                for h in range(NH_LOC):
                    # previous chunk's outproj blocks: first half into the
                    # h0 j-loop, second half into h1's.
                    nh = len(prev_po)
                    pending = prev_po[:nh // 2] if h == 0 else prev_po[nh // 2:]
                    hsl = slice(h * HS, (h + 1) * HS)
                    yt = psB.tile([P, SQB], FP, tag="yt")
                    for j in range(SKC):
                        st = psA.tile([P, SQB], FP, tag="st")
                        for r in range(csz // 512):
                            rsl = slice(r * 512, (r + 1) * 512)
                            nc.tensor.matmul(
                                st[:, rsl],
                                lhsT=kF_sb[hsl, :, j * P:(j + 1) * P],
                                rhs=qF_sb[hsl, :, cq0 + r * 512:
                                          cq0 + (r + 1) * 512],
                                start=True,
                                stop=True,
                                perf_mode=DR,
                            )
                        pt = ptp.tile([P, SQB], BF, tag="pt")
                        nc.scalar.activation(
                            pt[:, :csz], st[:, :csz], AF.Exp, scale=0.125)
                        for r in range(csz // 512):
                            rsl = slice(r * 512, (r + 1) * 512)
                            nc.tensor.matmul(
                                yt[:HS + 1, rsl],
                                lhsT=vaug_sb[:, h, j, :],
                                rhs=pt[:, rsl],
                                start=(j == 0),
                                stop=(j == SKC - 1),
                            )
                        if pending and j >= SKC - 3 * len(pending) and \
                                (SKC - 1 - j) % 3 == 0:
                            emit_po(pending[(j - (SKC - 3 * len(pending)))
                                            // 3])
                    # normalize: y = yt[:64] * (1/Z) (Z in row 64); 1/Z on
                    # DVE, partition broadcast on gpsimd.  The very last
                    # (chunk, h) is the kernel tail: 4 pipelined splits,
                    # each chased by its outproj blocks.
                    last = (cq0, h) == (chunks[-1][0], NH_LOC - 1)
                    # small splits first: the first reciprocal gates the
                    # whole outproj tail, the big ones hide under it.
                    splits = [(0, 128), (128, 128), (256, 256),
                              (512, 512)] if last else [(0, 512), (512, 512)]
                    for (s0, w) in splits:
                        lsl = slice(s0, s0 + w)
                        gsl = slice(cq0 + s0, cq0 + s0 + w)
                        zri = nrm.tile([1, SQB], FP, tag="zri")
                        nc.vector.reciprocal(
                            zri[:, lsl], yt[HS:HS + 1, lsl])
                        zbs = nrm.tile([HS, SQB], FP, tag="zbs")
                        nc.gpsimd.partition_broadcast(
                            zbs[:, lsl], zri[:, lsl])
                        nc.vector.tensor_mul(
                            yTn_sb[hsl, gsl], yt[:HS, lsl], zbs[:, lsl])
                        if last:
                            for sc in range((cq0 + s0) // P,
                                            (cq0 + s0 + w) // P):
                                emit_po(sc, tail=True)
                prev_po = list(range(cq0 // P, (cq0 + csz) // P))

    nc.compile()
    return nc


_NC_CACHE = None


def _get_nc():
    global _NC_CACHE
    if _NC_CACHE is None:
        _NC_CACHE = build_nc()
    return _NC_CACHE


def make_in_maps(x, W_qkv, b_qkv, rotary, W_dense, b_dense):
    x = np.asarray(x, dtype=np.float32)
    W_qkv = np.asarray(W_qkv, dtype=np.float32)
    b_qkv = np.asarray(b_qkv, dtype=np.float32)
    rotary = np.asarray(rotary, dtype=np.float32)
    W_dense = np.asarray(W_dense, dtype=np.float32)

    bf16 = ml_dtypes.bfloat16
    xT = np.ascontiguousarray(x.reshape(S, E).T.astype(bf16))
    wq = W_qkv[0:E, :]            # [E(out f), E(in e)]
    bq = b_qkv[0:E]
    in_maps = []
    for c in range(N_CORES):
        lo, hi = P * c, P * (c + 1)
        rot_c = rotary[:, lo:hi]                    # [E(f), 128(g)]
        wqT_eff = wq.T @ rot_c                      # [E(e), 128(g)]
        bqe = bq @ rot_c                            # [128(g)]
        in_maps.append({
            "xT": xT,
            "wqT": np.ascontiguousarray(wqT_eff.astype(bf16)),
            "wkT": np.ascontiguousarray(W_qkv[E + lo:E + hi, :].T.astype(bf16)),
            "wvT": np.ascontiguousarray(
                W_qkv[2 * E + lo:2 * E + hi, :].T.astype(bf16)),
            "wdT": np.ascontiguousarray(W_dense[:, lo:hi].T.astype(bf16)),
            "bqe": np.ascontiguousarray(bqe),
            "bk": np.ascontiguousarray(b_qkv[E + lo:E + hi]),
            "bv": np.ascontiguousarray(b_qkv[2 * E + lo:2 * E + hi]),
        })
    return in_maps


def run(inputs, trace=False, **trace_kwargs):
    """Run on 8 cores; returns (full_output, BassKernelResults)."""
    nc = _get_nc()
    in_maps = make_in_maps(**inputs)
    br = run_bass_kernel_spmd(
        nc, in_maps, core_ids=list(range(N_CORES)), trace=trace, **trace_kwargs
    )
    b_dense = np.asarray(inputs["b_dense"], dtype=np.float32)
    acc = np.zeros((S, E), dtype=np.float32)
    for r in br.results:
        acc += np.asarray(r["out"], dtype=np.float32)
    acc += b_dense[None, :]
    return acc[None, :, :], br


def kernel(**inputs) -> np.ndarray:
    out, _ = run(inputs, trace=False)
    return out
